# revision 37
# baseline (speedup 1.0000x reference)
"""GNN NodeBlock (segment_sum scatter + 2-layer MLP) on 8 Trainium2 cores.

Memory-bound regime -> the kernel is built around halving HBM traffic with
fp8, then keeping all five engines off the critical path:

 - 2 graphs x 4 cores; each core owns a 12500-node range (98 blocks of
   128 nodes) and all edges whose receiver falls in it.
 - Edge features are quantized to fp8 e4m3 host-side with per-receiver
   error feedback (error diffusion): within each receiver's edge list the
   running quantization error is carried into the next edge, so the
   on-device fp32 segment sum of fp8 values tracks the fp32 sum to ~1 ulp.
   End-to-end rel err ~9e-3 vs ~2.2e-2 for plain fp8 rounding.
 - Host buckets edges by 32-node stripe (4 stripes/block, 3 chunks of 128
   edges per stripe). Overflowing stripes (>384 edges, ~0.1%) are
   pre-compressed by summing duplicate receivers host-side.
 - One flat u8 DRAM stream per core, per-slab (8 blocks) layout
   [edge payload 8x1536 | node_T fp8 8x128 | ids bf16 8x24], so each slab
   is a single 13.5KB-per-partition DMA (large packets ~27GB/s/engine).
   Slabs are prefetched 3 deep; ids land first so the one-hot build never
   waits on the payload body.
 - Device, per slab: one wide DVE is_equal builds the bf16 one-hot
   oh[p, (block,stripe), chunk, node] (node innermost -> contiguous
   matmul rhs; a strided rhs costs ~3x on the PE). Scatter runs as
   mixed-dtype matmuls: fp8 payload chunk stationary (double-rate
   LDWEIGHTS) x bf16 one-hot moving -> feature-major agg in PSUM.
   DoubleRow fp8 was measured slower (LDW-bound) and is not used.
 - MLP feature-major over groups of 4 blocks, bf16 weights:
   h = relu(W1.T @ [agg; node_T] + b1) (node rhs read as fp8 straight
   from the stream tile), out_T = W2.T @ h + b2. agg cast on scalar,
   relu on scalar, bias-add on vector, out DMA on the scalar queue right
   after the producer so no queue ever stalls another slab's work.
 - All scatters of a slab are issued before its MLPs so the scalar cast
   overlaps the next group's scatter matmuls.
"""
import numpy as np
import ml_dtypes as md

import concourse.bacc as bacc
import concourse.mybir as mybir
from concourse.tile import TileContext
from concourse.bass_utils import run_bass_kernel_spmd

B, N, E = 2, 50000, 512000
De, Dv, H, Do = 128, 128, 256, 128
NCORES = 8
CPG = 4                    # cores per graph
NPC = N // CPG             # 12500 nodes per core
NB = 98                    # 128-node blocks per core
SW = 32                    # stripe width (nodes per one-hot stripe)
NST = 4                    # stripes per block
CPS = 3                    # chunks per stripe
SCAP = CPS * 128           # edge capacity per stripe
NPCH = 12                  # physical 128-edge chunks per block
NSC = 12                   # logical chunks (3 per stripe, no sharing)
PAYW = NPCH * 128          # 1536 payload bytes per block per partition
SLAB = 8                   # blocks per DMA slab
SLABS = [8] * (NB // SLAB) + ([NB % SLAB] if NB % SLAB else [])   # 12x8 + [2]
GRP = 4                    # blocks per MLP group
BLKW = PAYW + 128 + 24     # u8 stream bytes per block: payload + nodes + ids

F32 = mybir.dt.float32
BF16 = mybir.dt.bfloat16
FP8 = mybir.dt.float8e4
U8 = mybir.dt.uint8
NPF8 = md.float8_e4m3
NPBF = md.bfloat16


def _build_nc():
    nc = bacc.Bacc("TRN2", target_bir_lowering=False)
    stream = nc.dram_tensor("stream", [128, NB * BLKW], U8, kind="ExternalInput")
    cstd = nc.dram_tensor("cstd", [128, 1612], U8, kind="ExternalInput")
    out_g = nc.dram_tensor("out_g", [128, NB * 128], BF16, kind="ExternalOutput")

    with TileContext(nc) as tc:
        with tc.tile_pool(name="const", bufs=1) as cp, \
             tc.tile_pool(name="pay", bufs=5) as payp, \
             tc.tile_pool(name="oh", bufs=5) as ohp, \
             tc.tile_pool(name="agg", bufs=3) as aggp, \
             tc.tile_pool(name="hsb", bufs=3) as hp, \
             tc.tile_pool(name="osb", bufs=3) as op_, \
             tc.tile_pool(name="psA", bufs=3, space="PSUM") as psA, \
             tc.tile_pool(name="psH", bufs=2, space="PSUM") as psH, \
             tc.tile_pool(name="psO", bufs=1, space="PSUM") as psO:
            cst = cp.tile([128, 1612], U8)
            nc.sync.dma_start(out=cst[:], in_=cstd[:, :])
            w1_sb = cst[:, 0:1024].bitcast(BF16)       # [128, 512]
            w2_sb = cst[:, 1024:1536].bitcast(BF16)    # [128, 256]
            b1_sb = cst[:, 1536:1544].bitcast(F32)     # [128, 2]
            b2_sb = cst[:, 1544:1548].bitcast(F32)     # [128, 1]
            iota_sb = cst[:, 1548:1612].bitcast(BF16)  # [128, 32]

            slab_tiles = {}

            def issue_slab_load(si):
                """DMA a slab's u8 stream and build its one-hot (DVE).
                Issued one slab ahead so the DVE op is queued before the
                current slab's dependent work and the PE never waits."""
                nb_s = SLABS[si]
                pay = payp.tile([128, SLAB * BLKW], U8, name="payt")
                base = sum(SLABS[:si]) * BLKW
                ido = nb_s * (PAYW + 128)   # ids offset in slab (even)
                nc.gpsimd.dma_start(out=pay[:, ido:ido + nb_s * 24],
                                     in_=stream[:, base + ido:base + nb_s * BLKW])
                eng = nc.sync if si % 2 == 0 else nc.gpsimd
                eng.dma_start(out=pay[:, :ido],
                              in_=stream[:, base:base + ido])
                nq = NST * nb_s             # stripes in slab
                ids_ap = pay[:, ido:ido + nb_s * 24].bitcast(BF16)
                # one-hot for the whole slab in one DVE op:
                # oh[p, q, c, n] = (ids[p, q, c] == n), q = 4*block + stripe
                oh = ohp.tile([128, SLAB * NST * CPS * SW], BF16, name="oht")
                nc.vector.tensor_tensor(
                    out=oh[:, :nq * CPS * SW].rearrange(
                        "p (q c n) -> p q c n", q=nq, c=CPS),
                    in0=ids_ap.rearrange("p (q c) -> p q c", c=CPS)
                        .unsqueeze(3).broadcast_to([128, nq, CPS, SW]),
                    in1=iota_sb[:].unsqueeze(1).unsqueeze(1)
                        .broadcast_to([128, nq, CPS, SW]),
                    op=mybir.AluOpType.is_equal,
                )
                slab_tiles[si] = (pay, oh)

            issue_slab_load(0)
            issue_slab_load(1)
            issue_slab_load(2)
            for si, nb_s in enumerate(SLABS):
                if si + 3 < len(SLABS):
                    issue_slab_load(si + 3)
                pay, oh = slab_tiles.pop(si)
                nodo = nb_s * PAYW          # nodes offset in slab
                aggs = []
                for g0 in range(0, nb_s, GRP):
                    g_sz = min(GRP, nb_s - g0)
                    agg_ps = psA.tile([128, GRP * 128], F32, space="PSUM",
                                      name="agg_ps")
                    for b in range(g0, g0 + g_sz):
                        for s in range(NST):
                            q = NST * b + s
                            col = (b - g0) * 128 + s * SW
                            for c in range(CPS):
                                pc = CPS * s + c
                                nc.tensor.matmul(
                                    out=agg_ps[:, col:col + SW],
                                    lhsT=pay[:, b * PAYW + pc * 128:
                                             b * PAYW + (pc + 1) * 128].bitcast(FP8),
                                    rhs=oh[:, (q * CPS + c) * SW:
                                           (q * CPS + c + 1) * SW],
                                    start=(c == 0), stop=(c == CPS - 1),
                                )
                    aggs.append((g0, g_sz, agg_ps))
                for g0, g_sz, agg_ps in aggs:
                    agg_sb = aggp.tile([128, GRP * 128], BF16, name="agg_sb")
                    nc.scalar.copy(agg_sb[:, :g_sz * 128],
                                   agg_ps[:, :g_sz * 128])

                    nod = pay[:, nodo + g0 * 128:
                              nodo + (g0 + g_sz) * 128].bitcast(FP8)
                    hps = psH.tile([128, 2 * GRP * 128], F32, space="PSUM",
                                   name="hps")
                    for hm in range(2):
                        for dk in range(2):
                            nc.tensor.matmul(
                                out=hps[:, hm * GRP * 128:
                                        hm * GRP * 128 + g_sz * 128],
                                lhsT=w1_sb[:, dk * 256 + hm * 128:
                                           dk * 256 + (hm + 1) * 128],
                                rhs=(agg_sb[:, :g_sz * 128] if dk == 0 else nod),
                                start=(dk == 0), stop=(dk == 1),
                            )
                    h_sb = hp.tile([128, 2 * GRP * 128], BF16, name="h_sb")
                    for hm in range(2):
                        nc.scalar.activation(
                            out=h_sb[:, hm * GRP * 128:
                                     hm * GRP * 128 + g_sz * 128],
                            in_=hps[:, hm * GRP * 128:
                                    hm * GRP * 128 + g_sz * 128],
                            func=mybir.ActivationFunctionType.Relu,
                            bias=b1_sb[:, hm:hm + 1],
                        )
                    ops = psO.tile([128, GRP * 128], F32, space="PSUM",
                                   name="ops")
                    for hm in range(2):
                        nc.tensor.matmul(
                            out=ops[:, :g_sz * 128],
                            lhsT=w2_sb[:, hm * 128:(hm + 1) * 128],
                            rhs=h_sb[:, hm * GRP * 128:
                                     hm * GRP * 128 + g_sz * 128],
                            start=(hm == 0), stop=(hm == 1),
                        )
                    if g0 == 0:
                        o_sb = op_.tile([128, SLAB * 128], BF16, name="o_sb")
                    nc.vector.tensor_scalar_add(
                        out=o_sb[:, g0 * 128:(g0 + g_sz) * 128],
                        in0=ops[:, :g_sz * 128],
                        scalar1=b2_sb[:, 0:1],
                    )
                ob = sum(SLABS[:si]) * 128
                nc.scalar.dma_start(out=out_g[:, ob:ob + nb_s * 128],
                                    in_=o_sb[:, :nb_s * 128])
    nc.compile()
    return nc


def _feedback_quant(edge, rid):
    """Quantize [E, De] f32 to fp8 e4m3 with per-(receiver, feature) error
    diffusion so each receiver's segment sum is preserved to ~1 ulp."""
    ne = len(rid)
    order = np.argsort(rid, kind="stable")
    x = edge[order]
    r = rid[order]
    starts = np.r_[0, np.nonzero(np.diff(r))[0] + 1]
    lens = np.diff(np.r_[starts, ne])
    kmax = lens.max()
    pos = np.arange(ne) - np.repeat(starts, lens)
    idx_of = np.full((len(starts), kmax), -1, np.int64)
    idx_of[np.repeat(np.arange(len(starts)), lens), pos] = np.arange(ne)
    carry = np.zeros((len(starts), De), np.float32)
    q = np.empty((ne, De), NPF8)
    for k in range(kmax):
        sel = idx_of[:, k] >= 0
        ii = idx_of[sel, k]
        t = x[ii] + carry[sel]
        qq = t.astype(NPF8)
        q[ii] = qq
        carry[sel] = t - qq.astype(np.float32)
    out = np.empty_like(q)
    out[order] = q
    return out


def _prep_core(efeat, blk, within, node8):
    """Build one core's flat u8 stream from its edges.

    efeat: [ne, De] fp8 (feedback-quantized); blk/within: local block and
    within-block node id per edge; node8: [128, NB*128] fp8 node features
    (feature-major, padded)."""
    half = within // SW
    stripe = blk * NST + half
    local = within - half * SW
    nstripes = NB * NST
    counts = np.bincount(stripe, minlength=nstripes)
    if (counts > SCAP).any():
        # tail-fold overflowing stripes: sum the last (c-640) edges by
        # receiver (<=64 rows), keeping total <= 640 + 64 = SCAP
        keep = np.ones(len(stripe), bool)
        extra_f, extra_s, extra_l = [], [], []
        for ob in np.nonzero(counts > SCAP)[0]:
            idxs = np.nonzero(stripe == ob)[0]
            tail = idxs[SCAP - SW:]
            keep[tail] = False
            seg = np.zeros((SW, De), np.float32)
            np.add.at(seg, local[tail], efeat[tail].astype(np.float32))
            rows = np.unique(local[tail])
            extra_f.append(seg[rows].astype(NPF8))
            extra_s.append(np.full(len(rows), ob, stripe.dtype))
            extra_l.append(rows.astype(local.dtype))
        efeat = np.concatenate([efeat[keep]] + extra_f)
        stripe = np.concatenate([stripe[keep]] + extra_s)
        local = np.concatenate([local[keep]] + extra_l)
        counts = np.bincount(stripe, minlength=nstripes)

    order = np.argsort(stripe, kind="stable")
    str_s = stripe[order]
    offs = np.zeros(nstripes, np.int64)
    np.cumsum(counts[:-1], out=offs[1:])
    pos = np.arange(len(str_s)) - offs[str_s]
    blk_s = str_s // NST
    h_s = str_s % NST
    pc = CPS * h_s + pos // 128
    prow = pos % 128
    idc = pc

    feat = np.zeros((NB, 128, NPCH, 128), NPF8)
    feat[blk_s, prow, pc, :] = efeat[order]
    pay_b = feat.view(np.uint8).transpose(1, 0, 2, 3).reshape(128, NB, PAYW)

    ids = np.full((NB, 128, NSC), float(SW), np.float32)
    ids[blk_s, prow, idc] = local[order]
    ids_b = (np.ascontiguousarray(ids.transpose(1, 0, 2)).astype(NPBF)
             .view(np.uint8).reshape(128, NB, 24))

    stream = np.empty((128, NB * BLKW), np.uint8)
    for si, nb_s in enumerate(SLABS):
        o = sum(SLABS[:si]) * BLKW
        b0 = sum(SLABS[:si])
        stream[:, o:o + nb_s * PAYW] = \
            pay_b[:, b0:b0 + nb_s].reshape(128, -1)
        stream[:, o + nb_s * PAYW:o + nb_s * (PAYW + 128)] = \
            node8[:, b0 * 128:(b0 + nb_s) * 128]
        stream[:, o + nb_s * (PAYW + 128):o + nb_s * BLKW] = \
            ids_b[:, b0:b0 + nb_s].reshape(128, -1)
    return {"stream": stream}


def kernel(edge_data, node_data, W1, b1, W2, b2, receiver_ids, _trace=False):
    edge_data = np.asarray(edge_data, np.float32)
    node_data = np.asarray(node_data, np.float32)
    W1 = np.asarray(W1, np.float32)
    b1 = np.asarray(b1, np.float32)
    W2 = np.asarray(W2, np.float32)
    b2 = np.asarray(b2, np.float32)
    rid = np.asarray(receiver_ids).astype(np.int64)

    w1_dev = np.ascontiguousarray(
        W1.reshape(2, 128, H).transpose(1, 0, 2).reshape(128, 2 * H)).astype(NPBF)
    w2_dev = np.ascontiguousarray(
        W2.reshape(2, 128, Do).transpose(1, 0, 2).reshape(128, 2 * Do)).astype(NPBF)
    b1_dev = np.ascontiguousarray(b1.reshape(2, 128).T.astype(np.float32))
    b2_dev = np.ascontiguousarray(b2.reshape(128, 1).astype(np.float32))
    iotax = np.ascontiguousarray(np.broadcast_to(
        np.arange(SW, dtype=np.float32)[None, :], (128, SW))).astype(NPBF)
    cstd = np.concatenate([
        w1_dev.view(np.uint8), w2_dev.view(np.uint8),
        b1_dev.view(np.uint8), b2_dev.view(np.uint8),
        iotax.view(np.uint8)], axis=1)
    assert cstd.shape == (128, 1612), cstd.shape

    equant = [_feedback_quant(edge_data[g], rid[g]) for g in range(B)]

    in_maps = []
    for core in range(NCORES):
        g, part = divmod(core, CPG)
        base = part * NPC
        sel = (rid[g] >= base) & (rid[g] < base + NPC)
        local = rid[g][sel] - base
        blk = local // 128
        within = local - blk * 128
        efeat = equant[g][sel]

        nd = np.zeros((NB * 128, Dv), np.float32)
        nd[:NPC] = node_data[g, base:base + NPC]
        node8 = np.ascontiguousarray(nd.T).astype(NPF8).view(np.uint8)

        m = _prep_core(efeat, blk, within, node8)
        m.update({"cstd": cstd})
        in_maps.append(m)

    nc = _build_nc()
    res = run_bass_kernel_spmd(nc, in_maps, core_ids=list(range(NCORES)),
                               trace=_trace)

    out = np.empty((B, N, Do), np.float32)
    for core in range(NCORES):
        g, part = divmod(core, CPG)
        og = res.results[core]["out_g"].astype(np.float32)  # [128 o, NB*128]
        out[g, part * NPC:(part + 1) * NPC] = og.T[:NPC]
    if _trace:
        kernel._last = res
    return out


# revision 38
# speedup vs baseline: 1.1085x; 1.1085x over previous
"""GNN NodeBlock (segment_sum scatter + 2-layer MLP) on 8 Trainium2 cores.

Memory-bound regime -> the kernel is built around halving HBM traffic with
fp8, then keeping all five engines off the critical path:

 - 2 graphs x 4 cores; each core owns a 12500-node range (98 blocks of
   128 nodes) and all edges whose receiver falls in it.
 - Edge features are quantized to fp8 e4m3 host-side with per-receiver
   error feedback (error diffusion): within each receiver's edge list the
   running quantization error is carried into the next edge, so the
   on-device fp32 segment sum of fp8 values tracks the fp32 sum to ~1 ulp.
   End-to-end rel err ~9e-3 vs ~2.2e-2 for plain fp8 rounding.
 - Host buckets edges by 32-node stripe (4 stripes/block, 3 chunks of 128
   edges per stripe). Overflowing stripes (>384 edges, ~0.1%) are
   pre-compressed by summing duplicate receivers host-side.
 - One flat u8 DRAM stream per core, per-slab (8 blocks) layout
   [edge payload 8x1536 | node_T fp8 8x128 | ids bf16 8x24], so each slab
   is a single 13.5KB-per-partition DMA (large packets ~27GB/s/engine).
   Slabs are prefetched 3 deep; ids land first so the one-hot build never
   waits on the payload body.
 - Device, per slab: one wide DVE is_equal builds the bf16 one-hot
   oh[p, (block,stripe), chunk, node] (node innermost -> contiguous
   matmul rhs; a strided rhs costs ~3x on the PE). Scatter runs as
   mixed-dtype matmuls: fp8 payload chunk stationary (double-rate
   LDWEIGHTS) x bf16 one-hot moving -> feature-major agg in PSUM.
   DoubleRow fp8 was measured slower (LDW-bound) and is not used.
 - MLP feature-major over groups of 4 blocks, bf16 weights:
   h = relu(W1.T @ [agg; node_T] + b1) (node rhs read as fp8 straight
   from the stream tile), out_T = W2.T @ h + b2. agg cast on scalar,
   relu on scalar, bias-add on vector, out DMA on the scalar queue right
   after the producer so no queue ever stalls another slab's work.
 - All scatters of a slab are issued before its MLPs so the scalar cast
   overlaps the next group's scatter matmuls.
"""
import numpy as np
import ml_dtypes as md

import concourse.bacc as bacc
import concourse.mybir as mybir
from concourse.tile import TileContext
from concourse.bass_utils import run_bass_kernel_spmd

B, N, E = 2, 50000, 512000
De, Dv, H, Do = 128, 128, 256, 128
NCORES = 8
CPG = 4                    # cores per graph
NPC = N // CPG             # 12500 nodes per core
NB = 98                    # 128-node blocks per core
SW = 32                    # stripe width (nodes per one-hot stripe)
NST = 4                    # stripes per block
CPS = 3                    # chunks per stripe
SCAP = CPS * 128           # edge capacity per stripe
NPCH = 12                  # physical 128-edge chunks per block
NSC = 12                   # logical chunks (3 per stripe, no sharing)
PAYW = NPCH * 128          # 1536 payload bytes per block per partition
SLAB = 8                   # blocks per DMA slab
SLABS = [8] * (NB // SLAB) + ([NB % SLAB] if NB % SLAB else [])   # 12x8 + [2]
GRP = 4                    # blocks per MLP group
BLKW = PAYW + 128 + 24     # u8 stream bytes per block: payload + nodes + ids

F32 = mybir.dt.float32
BF16 = mybir.dt.bfloat16
FP8 = mybir.dt.float8e4
U8 = mybir.dt.uint8
NPF8 = md.float8_e4m3
NPBF = md.bfloat16


def _build_nc():
    nc = bacc.Bacc("TRN2", target_bir_lowering=False)
    stream = nc.dram_tensor("stream", [128, NB * BLKW], U8, kind="ExternalInput")
    cstd = nc.dram_tensor("cstd", [128, 1612], U8, kind="ExternalInput")
    out_g = nc.dram_tensor("out_g", [128, NB * 128], BF16, kind="ExternalOutput")

    with TileContext(nc) as tc:
        with tc.tile_pool(name="const", bufs=1) as cp, \
             tc.tile_pool(name="pay", bufs=5) as payp, \
             tc.tile_pool(name="oh", bufs=5) as ohp, \
             tc.tile_pool(name="agg", bufs=3) as aggp, \
             tc.tile_pool(name="hsb", bufs=3) as hp, \
             tc.tile_pool(name="osb", bufs=3) as op_, \
             tc.tile_pool(name="psA", bufs=2, space="PSUM") as psA, \
             tc.tile_pool(name="psH", bufs=2, space="PSUM") as psH, \
             tc.tile_pool(name="psO", bufs=2, space="PSUM") as psO:
            cst = cp.tile([128, 1612], U8)
            nc.sync.dma_start(out=cst[:], in_=cstd[:, :])
            w1_sb = cst[:, 0:1024].bitcast(BF16)       # [128, 512]
            w2_sb = cst[:, 1024:1536].bitcast(BF16)    # [128, 256]
            b1_sb = cst[:, 1536:1544].bitcast(F32)     # [128, 2]
            b2_sb = cst[:, 1544:1548].bitcast(F32)     # [128, 1]
            iota_sb = cst[:, 1548:1612].bitcast(BF16)  # [128, 32]

            slab_tiles = {}

            def issue_slab_load(si):
                """DMA a slab's u8 stream and build its one-hot (DVE).
                Issued one slab ahead so the DVE op is queued before the
                current slab's dependent work and the PE never waits."""
                nb_s = SLABS[si]
                pay = payp.tile([128, SLAB * BLKW], U8, name="payt")
                base = sum(SLABS[:si]) * BLKW
                ido = nb_s * (PAYW + 128)   # ids offset in slab (even)
                nc.gpsimd.dma_start(out=pay[:, ido:ido + nb_s * 24],
                                     in_=stream[:, base + ido:base + nb_s * BLKW])
                eng = nc.sync if si % 2 == 0 else nc.gpsimd
                eng.dma_start(out=pay[:, :ido],
                              in_=stream[:, base:base + ido])
                nq = NST * nb_s             # stripes in slab
                ids_ap = pay[:, ido:ido + nb_s * 24].bitcast(BF16)
                # one-hot for the whole slab in one DVE op:
                # oh[p, q, c, n] = (ids[p, q, c] == n), q = 4*block + stripe
                oh = ohp.tile([128, SLAB * NST * CPS * SW], BF16, name="oht")
                nc.vector.tensor_tensor(
                    out=oh[:, :nq * CPS * SW].rearrange(
                        "p (q c n) -> p q c n", q=nq, c=CPS),
                    in0=ids_ap.rearrange("p (q c) -> p q c", c=CPS)
                        .unsqueeze(3).broadcast_to([128, nq, CPS, SW]),
                    in1=iota_sb[:].unsqueeze(1).unsqueeze(1)
                        .broadcast_to([128, nq, CPS, SW]),
                    op=mybir.AluOpType.is_equal,
                )
                slab_tiles[si] = (pay, oh)

            issue_slab_load(0)
            issue_slab_load(1)
            issue_slab_load(2)
            for si, nb_s in enumerate(SLABS):
                if si + 3 < len(SLABS):
                    issue_slab_load(si + 3)
                pay, oh = slab_tiles.pop(si)
                nodo = nb_s * PAYW          # nodes offset in slab
                aggs = []
                for g0 in range(0, nb_s, GRP):
                    g_sz = min(GRP, nb_s - g0)
                    agg_ps = psA.tile([128, GRP * 128], F32, space="PSUM",
                                      name="agg_ps")
                    for b in range(g0, g0 + g_sz):
                        for s in range(NST):
                            q = NST * b + s
                            col = (b - g0) * 128 + s * SW
                            for c in range(CPS):
                                pc = CPS * s + c
                                nc.tensor.matmul(
                                    out=agg_ps[:, col:col + SW],
                                    lhsT=pay[:, b * PAYW + pc * 128:
                                             b * PAYW + (pc + 1) * 128].bitcast(FP8),
                                    rhs=oh[:, (q * CPS + c) * SW:
                                           (q * CPS + c + 1) * SW],
                                    start=(c == 0), stop=(c == CPS - 1),
                                )
                    aggs.append((g0, g_sz, agg_ps))
                for g0, g_sz, agg_ps in aggs:
                    agg_sb = aggp.tile([128, GRP * 128], BF16, name="agg_sb")
                    nc.scalar.copy(agg_sb[:, :g_sz * 128],
                                   agg_ps[:, :g_sz * 128])

                    nod = pay[:, nodo + g0 * 128:
                              nodo + (g0 + g_sz) * 128].bitcast(FP8)
                    hps = psH.tile([128, 2 * GRP * 128], F32, space="PSUM",
                                   name="hps")
                    for hm in range(2):
                        for dk in range(2):
                            nc.tensor.matmul(
                                out=hps[:, hm * GRP * 128:
                                        hm * GRP * 128 + g_sz * 128],
                                lhsT=w1_sb[:, dk * 256 + hm * 128:
                                           dk * 256 + (hm + 1) * 128],
                                rhs=(agg_sb[:, :g_sz * 128] if dk == 0 else nod),
                                start=(dk == 0), stop=(dk == 1),
                            )
                    h_sb = hp.tile([128, 2 * GRP * 128], BF16, name="h_sb")
                    for hm in range(2):
                        nc.scalar.activation(
                            out=h_sb[:, hm * GRP * 128:
                                     hm * GRP * 128 + g_sz * 128],
                            in_=hps[:, hm * GRP * 128:
                                    hm * GRP * 128 + g_sz * 128],
                            func=mybir.ActivationFunctionType.Relu,
                            bias=b1_sb[:, hm:hm + 1],
                        )
                    ops = psO.tile([128, GRP * 128], F32, space="PSUM",
                                   name="ops")
                    for hm in range(2):
                        nc.tensor.matmul(
                            out=ops[:, :g_sz * 128],
                            lhsT=w2_sb[:, hm * 128:(hm + 1) * 128],
                            rhs=h_sb[:, hm * GRP * 128:
                                     hm * GRP * 128 + g_sz * 128],
                            start=(hm == 0), stop=(hm == 1),
                        )
                    if g0 == 0:
                        o_sb = op_.tile([128, SLAB * 128], BF16, name="o_sb")
                    nc.vector.tensor_scalar_add(
                        out=o_sb[:, g0 * 128:(g0 + g_sz) * 128],
                        in0=ops[:, :g_sz * 128],
                        scalar1=b2_sb[:, 0:1],
                    )
                ob = sum(SLABS[:si]) * 128
                nc.scalar.dma_start(out=out_g[:, ob:ob + nb_s * 128],
                                    in_=o_sb[:, :nb_s * 128])
    nc.compile()
    return nc


def _feedback_quant(edge, rid):
    """Quantize [E, De] f32 to fp8 e4m3 with per-(receiver, feature) error
    diffusion so each receiver's segment sum is preserved to ~1 ulp."""
    ne = len(rid)
    order = np.argsort(rid, kind="stable")
    x = edge[order]
    r = rid[order]
    starts = np.r_[0, np.nonzero(np.diff(r))[0] + 1]
    lens = np.diff(np.r_[starts, ne])
    kmax = lens.max()
    pos = np.arange(ne) - np.repeat(starts, lens)
    idx_of = np.full((len(starts), kmax), -1, np.int64)
    idx_of[np.repeat(np.arange(len(starts)), lens), pos] = np.arange(ne)
    carry = np.zeros((len(starts), De), np.float32)
    q = np.empty((ne, De), NPF8)
    for k in range(kmax):
        sel = idx_of[:, k] >= 0
        ii = idx_of[sel, k]
        t = x[ii] + carry[sel]
        qq = t.astype(NPF8)
        q[ii] = qq
        carry[sel] = t - qq.astype(np.float32)
    out = np.empty_like(q)
    out[order] = q
    return out


def _prep_core(efeat, blk, within, node8):
    """Build one core's flat u8 stream from its edges.

    efeat: [ne, De] fp8 (feedback-quantized); blk/within: local block and
    within-block node id per edge; node8: [128, NB*128] fp8 node features
    (feature-major, padded)."""
    half = within // SW
    stripe = blk * NST + half
    local = within - half * SW
    nstripes = NB * NST
    counts = np.bincount(stripe, minlength=nstripes)
    if (counts > SCAP).any():
        # tail-fold overflowing stripes: sum the last (c-640) edges by
        # receiver (<=64 rows), keeping total <= 640 + 64 = SCAP
        keep = np.ones(len(stripe), bool)
        extra_f, extra_s, extra_l = [], [], []
        for ob in np.nonzero(counts > SCAP)[0]:
            idxs = np.nonzero(stripe == ob)[0]
            tail = idxs[SCAP - SW:]
            keep[tail] = False
            seg = np.zeros((SW, De), np.float32)
            np.add.at(seg, local[tail], efeat[tail].astype(np.float32))
            rows = np.unique(local[tail])
            extra_f.append(seg[rows].astype(NPF8))
            extra_s.append(np.full(len(rows), ob, stripe.dtype))
            extra_l.append(rows.astype(local.dtype))
        efeat = np.concatenate([efeat[keep]] + extra_f)
        stripe = np.concatenate([stripe[keep]] + extra_s)
        local = np.concatenate([local[keep]] + extra_l)
        counts = np.bincount(stripe, minlength=nstripes)

    order = np.argsort(stripe, kind="stable")
    str_s = stripe[order]
    offs = np.zeros(nstripes, np.int64)
    np.cumsum(counts[:-1], out=offs[1:])
    pos = np.arange(len(str_s)) - offs[str_s]
    blk_s = str_s // NST
    h_s = str_s % NST
    pc = CPS * h_s + pos // 128
    prow = pos % 128
    idc = pc

    feat = np.zeros((NB, 128, NPCH, 128), NPF8)
    feat[blk_s, prow, pc, :] = efeat[order]
    pay_b = feat.view(np.uint8).transpose(1, 0, 2, 3).reshape(128, NB, PAYW)

    ids = np.full((NB, 128, NSC), float(SW), np.float32)
    ids[blk_s, prow, idc] = local[order]
    ids_b = (np.ascontiguousarray(ids.transpose(1, 0, 2)).astype(NPBF)
             .view(np.uint8).reshape(128, NB, 24))

    stream = np.empty((128, NB * BLKW), np.uint8)
    for si, nb_s in enumerate(SLABS):
        o = sum(SLABS[:si]) * BLKW
        b0 = sum(SLABS[:si])
        stream[:, o:o + nb_s * PAYW] = \
            pay_b[:, b0:b0 + nb_s].reshape(128, -1)
        stream[:, o + nb_s * PAYW:o + nb_s * (PAYW + 128)] = \
            node8[:, b0 * 128:(b0 + nb_s) * 128]
        stream[:, o + nb_s * (PAYW + 128):o + nb_s * BLKW] = \
            ids_b[:, b0:b0 + nb_s].reshape(128, -1)
    return {"stream": stream}


def kernel(edge_data, node_data, W1, b1, W2, b2, receiver_ids, _trace=False):
    edge_data = np.asarray(edge_data, np.float32)
    node_data = np.asarray(node_data, np.float32)
    W1 = np.asarray(W1, np.float32)
    b1 = np.asarray(b1, np.float32)
    W2 = np.asarray(W2, np.float32)
    b2 = np.asarray(b2, np.float32)
    rid = np.asarray(receiver_ids).astype(np.int64)

    w1_dev = np.ascontiguousarray(
        W1.reshape(2, 128, H).transpose(1, 0, 2).reshape(128, 2 * H)).astype(NPBF)
    w2_dev = np.ascontiguousarray(
        W2.reshape(2, 128, Do).transpose(1, 0, 2).reshape(128, 2 * Do)).astype(NPBF)
    b1_dev = np.ascontiguousarray(b1.reshape(2, 128).T.astype(np.float32))
    b2_dev = np.ascontiguousarray(b2.reshape(128, 1).astype(np.float32))
    iotax = np.ascontiguousarray(np.broadcast_to(
        np.arange(SW, dtype=np.float32)[None, :], (128, SW))).astype(NPBF)
    cstd = np.concatenate([
        w1_dev.view(np.uint8), w2_dev.view(np.uint8),
        b1_dev.view(np.uint8), b2_dev.view(np.uint8),
        iotax.view(np.uint8)], axis=1)
    assert cstd.shape == (128, 1612), cstd.shape

    equant = [_feedback_quant(edge_data[g], rid[g]) for g in range(B)]

    in_maps = []
    for core in range(NCORES):
        g, part = divmod(core, CPG)
        base = part * NPC
        sel = (rid[g] >= base) & (rid[g] < base + NPC)
        local = rid[g][sel] - base
        blk = local // 128
        within = local - blk * 128
        efeat = equant[g][sel]

        nd = np.zeros((NB * 128, Dv), np.float32)
        nd[:NPC] = node_data[g, base:base + NPC]
        node8 = np.ascontiguousarray(nd.T).astype(NPF8).view(np.uint8)

        m = _prep_core(efeat, blk, within, node8)
        m.update({"cstd": cstd})
        in_maps.append(m)

    nc = _build_nc()
    res = run_bass_kernel_spmd(nc, in_maps, core_ids=list(range(NCORES)),
                               trace=_trace)

    out = np.empty((B, N, Do), np.float32)
    for core in range(NCORES):
        g, part = divmod(core, CPG)
        og = res.results[core]["out_g"].astype(np.float32)  # [128 o, NB*128]
        out[g, part * NPC:(part + 1) * NPC] = og.T[:NPC]
    if _trace:
        kernel._last = res
    return out


# revision 39
# speedup vs baseline: 1.1091x; 1.0006x over previous
"""GNN NodeBlock (segment_sum scatter + 2-layer MLP) on 8 Trainium2 cores.

Memory-bound regime -> the kernel is built around halving HBM traffic with
fp8, then keeping all five engines off the critical path:

 - 2 graphs x 4 cores; each core owns a 12500-node range (98 blocks of
   128 nodes) and all edges whose receiver falls in it.
 - Edge features are quantized to fp8 e4m3 host-side with per-receiver
   error feedback (error diffusion): within each receiver's edge list the
   running quantization error is carried into the next edge, so the
   on-device fp32 segment sum of fp8 values tracks the fp32 sum to ~1 ulp.
   End-to-end rel err ~9e-3 vs ~2.2e-2 for plain fp8 rounding.
 - Host buckets edges by 32-node stripe (4 stripes/block, 3 chunks of 128
   edges per stripe). Overflowing stripes (>384 edges, ~0.1%) are
   pre-compressed by summing duplicate receivers host-side.
 - One flat u8 DRAM stream per core, per-slab (8 blocks) layout
   [edge payload 8x1536 | node_T fp8 8x128 | ids bf16 8x24], so each slab
   is a single 13.5KB-per-partition DMA (large packets ~27GB/s/engine).
   Slabs are prefetched 3 deep; ids land first so the one-hot build never
   waits on the payload body.
 - Device, per slab: one wide DVE is_equal builds the bf16 one-hot
   oh[p, (block,stripe), chunk, node] (node innermost -> contiguous
   matmul rhs; a strided rhs costs ~3x on the PE). Scatter runs as
   mixed-dtype matmuls: fp8 payload chunk stationary (double-rate
   LDWEIGHTS) x bf16 one-hot moving -> feature-major agg in PSUM.
   DoubleRow fp8 was measured slower (LDW-bound) and is not used.
 - MLP feature-major over groups of 4 blocks, bf16 weights:
   h = relu(W1.T @ [agg; node_T] + b1) (node rhs read as fp8 straight
   from the stream tile), out_T = W2.T @ h + b2. agg cast on scalar,
   relu on scalar, bias-add on vector, out DMA on the scalar queue right
   after the producer so no queue ever stalls another slab's work.
 - All scatters of a slab are issued before its MLPs so the scalar cast
   overlaps the next group's scatter matmuls.
"""
import numpy as np
import ml_dtypes as md

import concourse.bacc as bacc
import concourse.mybir as mybir
from concourse.tile import TileContext
from concourse.bass_utils import run_bass_kernel_spmd

B, N, E = 2, 50000, 512000
De, Dv, H, Do = 128, 128, 256, 128
NCORES = 8
CPG = 4                    # cores per graph
NPC = N // CPG             # 12500 nodes per core
NB = 98                    # 128-node blocks per core
SW = 32                    # stripe width (nodes per one-hot stripe)
NST = 4                    # stripes per block
CPS = 3                    # chunks per stripe
SCAP = CPS * 128           # edge capacity per stripe
NPCH = 12                  # physical 128-edge chunks per block
NSC = 12                   # logical chunks (3 per stripe, no sharing)
PAYW = NPCH * 128          # 1536 payload bytes per block per partition
SLAB = 8                   # blocks per DMA slab
SLABS = [8] * (NB // SLAB) + ([NB % SLAB] if NB % SLAB else [])   # 12x8 + [2]
GRP = 4                    # blocks per MLP group
BLKW = PAYW + 128 + 24     # u8 stream bytes per block: payload + nodes + ids

F32 = mybir.dt.float32
BF16 = mybir.dt.bfloat16
FP8 = mybir.dt.float8e4
U8 = mybir.dt.uint8
NPF8 = md.float8_e4m3
NPBF = md.bfloat16


def _build_nc():
    nc = bacc.Bacc("TRN2", target_bir_lowering=False)
    stream = nc.dram_tensor("stream", [128, NB * BLKW], U8, kind="ExternalInput")
    cstd = nc.dram_tensor("cstd", [128, 1612], U8, kind="ExternalInput")
    out_g = nc.dram_tensor("out_g", [128, NB * 128], BF16, kind="ExternalOutput")

    with TileContext(nc) as tc:
        with tc.tile_pool(name="const", bufs=1) as cp, \
             tc.tile_pool(name="pay", bufs=5) as payp, \
             tc.tile_pool(name="oh", bufs=5) as ohp, \
             tc.tile_pool(name="agg", bufs=3) as aggp, \
             tc.tile_pool(name="hsb", bufs=3) as hp, \
             tc.tile_pool(name="osb", bufs=3) as op_, \
             tc.tile_pool(name="psA", bufs=2, space="PSUM") as psA, \
             tc.tile_pool(name="psH", bufs=2, space="PSUM") as psH, \
             tc.tile_pool(name="psO", bufs=2, space="PSUM") as psO:
            cst = cp.tile([128, 1612], U8)
            nc.sync.dma_start(out=cst[:], in_=cstd[:, :])
            w1_sb = cst[:, 0:1024].bitcast(BF16)       # [128, 512]
            w2_sb = cst[:, 1024:1536].bitcast(BF16)    # [128, 256]
            b1_sb = cst[:, 1536:1544].bitcast(F32)     # [128, 2]
            b2_sb = cst[:, 1544:1548].bitcast(F32)     # [128, 1]
            iota_sb = cst[:, 1548:1612].bitcast(BF16)  # [128, 32]

            slab_tiles = {}

            def issue_slab_load(si):
                """DMA a slab's u8 stream and build its one-hot (DVE).
                Issued one slab ahead so the DVE op is queued before the
                current slab's dependent work and the PE never waits."""
                nb_s = SLABS[si]
                pay = payp.tile([128, SLAB * BLKW], U8, name="payt")
                base = sum(SLABS[:si]) * BLKW
                ido = nb_s * (PAYW + 128)   # ids offset in slab (even)
                nc.gpsimd.dma_start(out=pay[:, ido:ido + nb_s * 24],
                                     in_=stream[:, base + ido:base + nb_s * BLKW])
                eng = nc.sync if si % 2 == 0 else nc.gpsimd
                eng.dma_start(out=pay[:, :ido],
                              in_=stream[:, base:base + ido])
                nq = NST * nb_s             # stripes in slab
                ids_ap = pay[:, ido:ido + nb_s * 24].bitcast(BF16)
                # one-hot for the whole slab in one DVE op:
                # oh[p, q, c, n] = (ids[p, q, c] == n), q = 4*block + stripe
                oh = ohp.tile([128, SLAB * NST * CPS * SW], BF16, name="oht")
                nc.vector.tensor_tensor(
                    out=oh[:, :nq * CPS * SW].rearrange(
                        "p (q c n) -> p q c n", q=nq, c=CPS),
                    in0=ids_ap.rearrange("p (q c) -> p q c", c=CPS)
                        .unsqueeze(3).broadcast_to([128, nq, CPS, SW]),
                    in1=iota_sb[:].unsqueeze(1).unsqueeze(1)
                        .broadcast_to([128, nq, CPS, SW]),
                    op=mybir.AluOpType.is_equal,
                )
                slab_tiles[si] = (pay, oh)

            issue_slab_load(0)
            issue_slab_load(1)
            issue_slab_load(2)
            for si, nb_s in enumerate(SLABS):
                if si + 3 < len(SLABS):
                    issue_slab_load(si + 3)
                pay, oh = slab_tiles.pop(si)
                nodo = nb_s * PAYW          # nodes offset in slab
                aggs = []
                for g0 in range(0, nb_s, GRP):
                    g_sz = min(GRP, nb_s - g0)
                    agg_ps = psA.tile([128, GRP * 128], F32, space="PSUM",
                                      name="agg_ps")
                    for b in range(g0, g0 + g_sz):
                        for s in range(NST):
                            q = NST * b + s
                            col = (b - g0) * 128 + s * SW
                            for c in range(CPS):
                                pc = CPS * s + c
                                nc.tensor.matmul(
                                    out=agg_ps[:, col:col + SW],
                                    lhsT=pay[:, b * PAYW + pc * 128:
                                             b * PAYW + (pc + 1) * 128].bitcast(FP8),
                                    rhs=oh[:, (q * CPS + c) * SW:
                                           (q * CPS + c + 1) * SW],
                                    start=(c == 0), stop=(c == CPS - 1),
                                )
                    aggs.append((g0, g_sz, agg_ps))
                for g0, g_sz, agg_ps in aggs:
                    agg_sb = aggp.tile([128, GRP * 128], BF16, name="agg_sb")
                    nc.scalar.copy(agg_sb[:, :g_sz * 128],
                                   agg_ps[:, :g_sz * 128])

                    nod = pay[:, nodo + g0 * 128:
                              nodo + (g0 + g_sz) * 128].bitcast(FP8)
                    hps = psH.tile([128, 2 * GRP * 128], F32, space="PSUM",
                                   name="hps")
                    for hm in range(2):
                        for dk in (1, 0):
                            nc.tensor.matmul(
                                out=hps[:, hm * GRP * 128:
                                        hm * GRP * 128 + g_sz * 128],
                                lhsT=w1_sb[:, dk * 256 + hm * 128:
                                           dk * 256 + (hm + 1) * 128],
                                rhs=(agg_sb[:, :g_sz * 128] if dk == 0 else nod),
                                start=(dk == 1), stop=(dk == 0),
                            )
                    h_sb = hp.tile([128, 2 * GRP * 128], BF16, name="h_sb")
                    for hm in range(2):
                        nc.scalar.activation(
                            out=h_sb[:, hm * GRP * 128:
                                     hm * GRP * 128 + g_sz * 128],
                            in_=hps[:, hm * GRP * 128:
                                    hm * GRP * 128 + g_sz * 128],
                            func=mybir.ActivationFunctionType.Relu,
                            bias=b1_sb[:, hm:hm + 1],
                        )
                    ops = psO.tile([128, GRP * 128], F32, space="PSUM",
                                   name="ops")
                    for hm in range(2):
                        nc.tensor.matmul(
                            out=ops[:, :g_sz * 128],
                            lhsT=w2_sb[:, hm * 128:(hm + 1) * 128],
                            rhs=h_sb[:, hm * GRP * 128:
                                     hm * GRP * 128 + g_sz * 128],
                            start=(hm == 0), stop=(hm == 1),
                        )
                    if g0 == 0:
                        o_sb = op_.tile([128, SLAB * 128], BF16, name="o_sb")
                    nc.vector.tensor_scalar_add(
                        out=o_sb[:, g0 * 128:(g0 + g_sz) * 128],
                        in0=ops[:, :g_sz * 128],
                        scalar1=b2_sb[:, 0:1],
                    )
                ob = sum(SLABS[:si]) * 128
                nc.scalar.dma_start(out=out_g[:, ob:ob + nb_s * 128],
                                    in_=o_sb[:, :nb_s * 128])
    nc.compile()
    return nc


def _feedback_quant(edge, rid):
    """Quantize [E, De] f32 to fp8 e4m3 with per-(receiver, feature) error
    diffusion so each receiver's segment sum is preserved to ~1 ulp."""
    ne = len(rid)
    order = np.argsort(rid, kind="stable")
    x = edge[order]
    r = rid[order]
    starts = np.r_[0, np.nonzero(np.diff(r))[0] + 1]
    lens = np.diff(np.r_[starts, ne])
    kmax = lens.max()
    pos = np.arange(ne) - np.repeat(starts, lens)
    idx_of = np.full((len(starts), kmax), -1, np.int64)
    idx_of[np.repeat(np.arange(len(starts)), lens), pos] = np.arange(ne)
    carry = np.zeros((len(starts), De), np.float32)
    q = np.empty((ne, De), NPF8)
    for k in range(kmax):
        sel = idx_of[:, k] >= 0
        ii = idx_of[sel, k]
        t = x[ii] + carry[sel]
        qq = t.astype(NPF8)
        q[ii] = qq
        carry[sel] = t - qq.astype(np.float32)
    out = np.empty_like(q)
    out[order] = q
    return out


def _prep_core(efeat, blk, within, node8):
    """Build one core's flat u8 stream from its edges.

    efeat: [ne, De] fp8 (feedback-quantized); blk/within: local block and
    within-block node id per edge; node8: [128, NB*128] fp8 node features
    (feature-major, padded)."""
    half = within // SW
    stripe = blk * NST + half
    local = within - half * SW
    nstripes = NB * NST
    counts = np.bincount(stripe, minlength=nstripes)
    if (counts > SCAP).any():
        # tail-fold overflowing stripes: sum the last (c-640) edges by
        # receiver (<=64 rows), keeping total <= 640 + 64 = SCAP
        keep = np.ones(len(stripe), bool)
        extra_f, extra_s, extra_l = [], [], []
        for ob in np.nonzero(counts > SCAP)[0]:
            idxs = np.nonzero(stripe == ob)[0]
            tail = idxs[SCAP - SW:]
            keep[tail] = False
            seg = np.zeros((SW, De), np.float32)
            np.add.at(seg, local[tail], efeat[tail].astype(np.float32))
            rows = np.unique(local[tail])
            extra_f.append(seg[rows].astype(NPF8))
            extra_s.append(np.full(len(rows), ob, stripe.dtype))
            extra_l.append(rows.astype(local.dtype))
        efeat = np.concatenate([efeat[keep]] + extra_f)
        stripe = np.concatenate([stripe[keep]] + extra_s)
        local = np.concatenate([local[keep]] + extra_l)
        counts = np.bincount(stripe, minlength=nstripes)

    order = np.argsort(stripe, kind="stable")
    str_s = stripe[order]
    offs = np.zeros(nstripes, np.int64)
    np.cumsum(counts[:-1], out=offs[1:])
    pos = np.arange(len(str_s)) - offs[str_s]
    blk_s = str_s // NST
    h_s = str_s % NST
    pc = CPS * h_s + pos // 128
    prow = pos % 128
    idc = pc

    feat = np.zeros((NB, 128, NPCH, 128), NPF8)
    feat[blk_s, prow, pc, :] = efeat[order]
    pay_b = feat.view(np.uint8).transpose(1, 0, 2, 3).reshape(128, NB, PAYW)

    ids = np.full((NB, 128, NSC), float(SW), np.float32)
    ids[blk_s, prow, idc] = local[order]
    ids_b = (np.ascontiguousarray(ids.transpose(1, 0, 2)).astype(NPBF)
             .view(np.uint8).reshape(128, NB, 24))

    stream = np.empty((128, NB * BLKW), np.uint8)
    for si, nb_s in enumerate(SLABS):
        o = sum(SLABS[:si]) * BLKW
        b0 = sum(SLABS[:si])
        stream[:, o:o + nb_s * PAYW] = \
            pay_b[:, b0:b0 + nb_s].reshape(128, -1)
        stream[:, o + nb_s * PAYW:o + nb_s * (PAYW + 128)] = \
            node8[:, b0 * 128:(b0 + nb_s) * 128]
        stream[:, o + nb_s * (PAYW + 128):o + nb_s * BLKW] = \
            ids_b[:, b0:b0 + nb_s].reshape(128, -1)
    return {"stream": stream}


def kernel(edge_data, node_data, W1, b1, W2, b2, receiver_ids, _trace=False):
    edge_data = np.asarray(edge_data, np.float32)
    node_data = np.asarray(node_data, np.float32)
    W1 = np.asarray(W1, np.float32)
    b1 = np.asarray(b1, np.float32)
    W2 = np.asarray(W2, np.float32)
    b2 = np.asarray(b2, np.float32)
    rid = np.asarray(receiver_ids).astype(np.int64)

    w1_dev = np.ascontiguousarray(
        W1.reshape(2, 128, H).transpose(1, 0, 2).reshape(128, 2 * H)).astype(NPBF)
    w2_dev = np.ascontiguousarray(
        W2.reshape(2, 128, Do).transpose(1, 0, 2).reshape(128, 2 * Do)).astype(NPBF)
    b1_dev = np.ascontiguousarray(b1.reshape(2, 128).T.astype(np.float32))
    b2_dev = np.ascontiguousarray(b2.reshape(128, 1).astype(np.float32))
    iotax = np.ascontiguousarray(np.broadcast_to(
        np.arange(SW, dtype=np.float32)[None, :], (128, SW))).astype(NPBF)
    cstd = np.concatenate([
        w1_dev.view(np.uint8), w2_dev.view(np.uint8),
        b1_dev.view(np.uint8), b2_dev.view(np.uint8),
        iotax.view(np.uint8)], axis=1)
    assert cstd.shape == (128, 1612), cstd.shape

    equant = [_feedback_quant(edge_data[g], rid[g]) for g in range(B)]

    in_maps = []
    for core in range(NCORES):
        g, part = divmod(core, CPG)
        base = part * NPC
        sel = (rid[g] >= base) & (rid[g] < base + NPC)
        local = rid[g][sel] - base
        blk = local // 128
        within = local - blk * 128
        efeat = equant[g][sel]

        nd = np.zeros((NB * 128, Dv), np.float32)
        nd[:NPC] = node_data[g, base:base + NPC]
        node8 = np.ascontiguousarray(nd.T).astype(NPF8).view(np.uint8)

        m = _prep_core(efeat, blk, within, node8)
        m.update({"cstd": cstd})
        in_maps.append(m)

    nc = _build_nc()
    res = run_bass_kernel_spmd(nc, in_maps, core_ids=list(range(NCORES)),
                               trace=_trace)

    out = np.empty((B, N, Do), np.float32)
    for core in range(NCORES):
        g, part = divmod(core, CPG)
        og = res.results[core]["out_g"].astype(np.float32)  # [128 o, NB*128]
        out[g, part * NPC:(part + 1) * NPC] = og.T[:NPC]
    if _trace:
        kernel._last = res
    return out
